# revision 33
# baseline (speedup 1.0000x reference)
"""Trainium2 Bass kernel for nn_DecoderOutLayer (per-frequency causal 5-tap
temporal conv: x[B,C,T,F], weight[F,C*5,O], b[F,O] -> out[B,O,T,F]).

Sharding: frequency axis F=96 split across 8 cores (12 freqs/core).

Per-core algorithm (all fp32):
  Host packs x as [fp=6, (f2,c)=128, b=4, t=1000]  (f-pair x channel on
  partitions).  Device:
   pass 1: for each (fp, b, t-window of 128): matmul with lhsT = X window
     (stationary, [128 part=(f2,c), 128 cols=t]) and rhs = W_exp[fp]
     [128, 20] where cols = (s=5 shifts, f2, o) and W_exp[(f2,c),(s,f2',o)]
     = delta(f2,f2') * w[f, c, 4-s, o].  psum1[t_local, (s,f2,o)] then
     holds the contribution of input time t to output time t + s.
   pass 2: shift-matrix matmuls S_s[t, t'] = delta(t' = t + 4 - s ... see
     below) contract over the 128 t-partitions to realign the 5 shifted
     contributions onto common output times, accumulating in PSUM, plus a
     rank-1 matmul that adds the bias.  Windows advance by 124 so every
     output column of a window is complete (no cross-window terms).
"""
import os
import sys

for _p in (
    "/root/.axon_site",
    "/root/.axon_site/_ro/trn_rl_repo",
    "/root/.axon_site/_ro/pypackages",
    "/opt/trn_rl_repo",
):
    if os.path.isdir(_p) and _p not in sys.path:
        sys.path.append(_p)

import numpy as np

import concourse.bass as bass
import concourse.mybir as mybir
import concourse.tile as tile
from concourse import bacc, bass_utils
from concourse.tile_rust import add_dep_helper

TC = 5
B, C, T, F = 4, 64, 1000, 96
O = 2
NCORES = 8
FL = F // NCORES       # 12 freqs per core
NFP = FL // 2          # 6 f-pairs
NTB = 9                # t-windows per b: 8 x stride-124 + tail at 876
WIN = 128
STEP = 124
TPAD = T + 4           # 1004 cols per b in SBUF (4 leading zeros)
NQ = 2 * O             # 4 = (f2, o)
PS2N = 2 * NFP * NTB * NQ   # 432 cols per psum2 bank (2 b's)
F32 = mybir.dt.float32


def _host_prep(x, weight, b):
    """Full inputs -> per-core input maps (numpy only)."""
    x = np.ascontiguousarray(np.asarray(x, dtype=np.float32))
    weight = np.asarray(weight, dtype=np.float32)
    bias = np.asarray(b, dtype=np.float32)
    x_t = np.ascontiguousarray(x.transpose(3, 1, 0, 2))   # [F,C,B,T]
    w4 = weight.reshape(F, C, TC, O)

    # shift matrices are core-independent: [128, 6*124] (partition-major)
    sm = np.zeros((6, 128, STEP), np.float32)
    for s in range(TC):
        tp = np.arange(STEP)
        sm[s, tp + 4 - s, tp] = 1.0
    sm[5, 0, :] = 1.0       # bias row-broadcast matrix
    sm_h = np.ascontiguousarray(sm.transpose(1, 0, 2).reshape(128, 6 * STEP))

    in_maps = []
    for g in range(NCORES):
        f0 = g * FL
        xg = np.zeros((NFP, 2 * C, B, TPAD), np.float32)
        xg[:, :, :, 4:] = x_t[f0:f0 + FL].reshape(NFP, 2 * C, B, T)
        # -> [3, 128, 2*B*TPAD]: two f-pairs per SBUF tile, partition-major
        xg = np.ascontiguousarray(
            xg.reshape(3, 2, 2 * C, B * TPAD).transpose(0, 2, 1, 3)
        ).reshape(3, 2 * C, 2 * B * TPAD)

        we = np.zeros((NFP, 128, 20), np.float32)
        w5 = w4[f0:f0 + FL].reshape(NFP, 2, C, TC, O)
        w5s = w5[:, :, :, ::-1, :]                        # tap k -> shift s=4-k
        wev = we.reshape(NFP, 2, C, TC, 2, O)             # fp,f2,c,s,f2',o
        for f2 in range(2):
            wev[:, f2, :, :, f2, :] = w5s[:, f2]

        br = np.zeros((128, PS2N), np.float32)
        v = bias[f0:f0 + FL].reshape(NFP, 2, O)
        arr = np.broadcast_to(v[:, None, :, :], (NFP, NTB, 2, O)).reshape(-1)
        br[0] = np.tile(arr, 2)

        consts = np.concatenate([
            we.transpose(1, 0, 2).reshape(128, NFP * 20), sm_h, br], axis=1)
        in_maps.append({
            "xin": np.ascontiguousarray(xg),
            "consts": np.ascontiguousarray(consts),
        })
    return in_maps


def build_program(nc):
    """Declare DRAM tensors + emit the Tile program. Returns out name."""
    xin = nc.dram_tensor("xin", [3, 128, 2 * B * TPAD], F32,
                         kind="ExternalInput").ap()
    ncst = NFP * 20 + 6 * STEP + PS2N      # 120 + 744 + 432 = 1296
    cst = nc.dram_tensor("consts", [128, ncst], F32, kind="ExternalInput").ap()
    # One DRAM output per b, raw [tb, p, q] window dump (the host drops the
    # tb=8 overlap rows). Separate tensors + exactly 8 HWDGE DMAs total in
    # the program: each DMA gets its own queue lane, so no DMA ever carries
    # more than one wait (walrus allows a single wait per instruction).
    outs = [nc.dram_tensor(f"out{bb}", [NTB * STEP, FL * O], F32,
                           kind="ExternalOutput").ap() for bb in range(B)]

    with tile.TileContext(nc) as tc:
        from contextlib import ExitStack
        with ExitStack() as ctx:
            const = ctx.enter_context(tc.tile_pool(name="const", bufs=1))
            ps1_pool = ctx.enter_context(
                tc.tile_pool(name="ps1", bufs=4, space="PSUM"))
            ps2_pool = ctx.enter_context(
                tc.tile_pool(name="ps2", bufs=2, space="PSUM"))

            cst_sb = const.tile([128, ncst], F32, name="cst_sb")
            tmp = const.tile([128, B * NFP * NTB * 20], F32, name="tmp")
            stage = const.tile([128, B * NTB * 24], F32, name="stage")
            xt = [const.tile([128, 2 * B * TPAD], F32, name=f"x_sb{j}")
                  for j in range(3)]
            w_sb = cst_sb[:, 0:NFP * 20]
            sm_sb = cst_sb[:, NFP * 20:NFP * 20 + 6 * STEP]
            br_sb = cst_sb[:, NFP * 20 + 6 * STEP:ncst]

            def xs(fp):
                return xt[fp // 2][:, (fp % 2) * B * TPAD:
                                   (fp % 2 + 1) * B * TPAD]

            # all constants in one DMA -> one wait semaphore for consumers
            nc.sync.dma_start(cst_sb[:], cst)

            # x in (pad zeros baked in host-side); 2 f-pairs per DMA keeps
            # the total HWDGE DMA count at 8 (one queue lane each)
            for j in range(3):
                nc.sync.dma_start(xt[j][:], xin[j])

            # This walrus build gives a Matmult a single sync-wait slot, so
            # each new DMA semaphore must be absorbed by a PE instruction
            # that needs no other wait: tiny [1,1] "touch" matmuls.
            scratch_pool = ctx.enter_context(
                tc.tile_pool(name="scratch", bufs=1, space="PSUM"))
            scratch = scratch_pool.tile([1, 1], F32, name="scratch")
            nc.tensor.matmul(scratch[:], lhsT=cst_sb[:, 0:1],
                             rhs=cst_sb[:, 0:1], start=True, stop=True)

            # pass 1
            PS1_BUFS = 4
            copy_insts = []
            lastmm_insts = []
            for fp in range(NFP):
                if fp % 2 == 0:
                    nc.tensor.matmul(scratch[:], lhsT=xs(fp)[:, 0:1],
                                     rhs=xs(fp)[:, 0:1], start=True, stop=True)
                for bb in range(B):
                    g = fp * B + bb
                    absorbers = []
                    if g >= PS1_BUFS:
                        # the recycled psum slot's release waits on two sems
                        # (PE: its matmuls, DVE: its copy); a Matmult can
                        # carry only ONE wait in this walrus build, so park
                        # each wait on its own dummy [1,1] matmul first
                        for dep in (lastmm_insts[g - PS1_BUFS],
                                    copy_insts[g - PS1_BUFS]):
                            dmm = nc.tensor.matmul(
                                scratch[:], lhsT=cst_sb[:, 0:1],
                                rhs=cst_sb[:, 0:1], start=True, stop=True)
                            add_dep_helper(dmm.ins, dep.ins, sync=True,
                                           reason="absorb slot-release wait")
                            absorbers.append(dmm)
                    ps1 = ps1_pool.tile([128, NTB * 20], F32, tag="ps1")
                    for tb in range(NTB):
                        c0 = tb * STEP if tb < 8 else TPAD - WIN
                        mm = nc.tensor.matmul(
                            ps1[:, tb * 20:(tb + 1) * 20],
                            lhsT=xs(fp)[:, bb * TPAD + c0: bb * TPAD + c0 + WIN],
                            rhs=w_sb[:, fp * 20:(fp + 1) * 20],
                            start=True, stop=True)
                        if tb == 0:
                            for dmm in absorbers:
                                add_dep_helper(mm.ins, dmm.ins, sync=False,
                                               reason="absorber before leader")
                        if tb == NTB - 1:
                            lastmm_insts.append(mm)
                    # single engine (DVE) so pass-2 consumers of `tmp` need
                    # only one semaphore wait (walrus wait-count limit)
                    ci = nc.vector.tensor_copy(
                        tmp[:, bb * 1080 + fp * 180: bb * 1080 + (fp + 1) * 180],
                        ps1[:])
                    copy_insts.append(ci)

            # pass 2: tmp col = b*1080 + (fp*9+tb)*20 + s*4 + (f2*2+o)
            tmp4 = tmp[:].rearrange("p (b m u) -> p b m u", b=B, m=NFP * NTB)
            for bank in range(2):
                ps2 = ps2_pool.tile([STEP, PS2N], F32, tag="ps2")
                nc.tensor.matmul(ps2[:], lhsT=sm_sb[:, 5 * STEP:6 * STEP],
                                 rhs=br_sb, start=True, stop=False)
                for s in range(TC):
                    rhs = tmp4[:, 2 * bank:2 * bank + 2, :, s * NQ:(s + 1) * NQ]
                    nc.tensor.matmul(ps2[:], lhsT=sm_sb[:, s * STEP:(s + 1) * STEP],
                                     rhs=rhs, start=False, stop=(s == TC - 1))
                # psum2 col = bl*216 + fp*36 + tb*4 + q -> stage col tb*24+fp*4+q
                for bl in range(2):
                    bb = 2 * bank + bl
                    src = ps2[:, bl * 216:(bl + 1) * 216].rearrange(
                        "p (f m q) -> p m f q", f=NFP, m=NTB)
                    nc.vector.tensor_copy(
                        stage[:STEP, bb * 216:(bb + 1) * 216].rearrange(
                            "p (m f q) -> p m f q", m=NTB, f=NFP), src)

            # out dump: [tb, p, q]; host maps t = tb*124 + p (tb<8) and
            # t = 876 + p (tb=8, p>=116), dropping overlap rows
            for bb in range(B):
                dst = outs[bb].rearrange("(m p) q -> p m q", p=STEP)
                nc.sync.dma_start(dst,
                                  stage[:STEP, bb * 216:(bb + 1) * 216]
                                  .rearrange("p (m q) -> p m q", m=NTB))
    return "out"


_CACHED = {}


def _get_nc():
    if "nc" not in _CACHED:
        # bacc.Bacc (not bass.Bass): its compile() pass legalizes multi-wait
        # instructions onto InstEventSemaphore (1 wait/inst ISA limit)
        nc = bacc.Bacc("TRN2", target_bir_lowering=False, debug=False,
                       num_devices=NCORES)
        build_program(nc)
        nc.compile()
        _CACHED["nc"] = nc
    return _CACHED["nc"]


def _gather(results):
    full = np.empty((B, O, T, F), np.float32)
    co = np.empty((B, T, FL * O), np.float32)
    for g in range(NCORES):
        for bb in range(B):
            arr = results[g][f"out{bb}"].reshape(NTB, STEP, FL * O)
            co[bb, :8 * STEP] = arr[:8].reshape(8 * STEP, FL * O)
            co[bb, 8 * STEP:] = arr[8, STEP - (T - 8 * STEP):]
        full[:, :, :, g * FL:(g + 1) * FL] = \
            co.reshape(B, T, FL, O).transpose(0, 3, 1, 2)
    return full


def kernel(x, weight, b, **run_kwargs):
    in_maps = _host_prep(x, weight, b)
    nc = _get_nc()
    res = bass_utils.run_bass_kernel_spmd(
        nc, in_maps, core_ids=list(range(NCORES)), **run_kwargs)
    out = _gather(res.results)
    if run_kwargs:
        return out, res
    return out


# revision 34
# speedup vs baseline: 3.1257x; 3.1257x over previous
"""Trainium2 Bass kernel for nn_DecoderOutLayer (per-frequency causal 5-tap
temporal conv: x[B,C,T,F], weight[F,C*5,O], b[F,O] -> out[B,O,T,F]).

Sharding: frequency axis F=96 split across 8 cores (12 freqs/core).

Per-core algorithm:
  Host packs x as [fp=6, (f2,c)=128, b=4, 4+t] (f-pair x channel on
  partitions, 4-zero causal pad baked in).  Device:
   pass 1: for each (fp, b, t-window of 128): matmul with lhsT = X window
     (stationary, [128 part=(f2,c), 128 cols=t]) and rhs = W_exp[fp]
     [128, 20] where cols = (s=5 shifts, f2, o) and W_exp[(f2,c),(s,f2',o)]
     = delta(f2,f2') * w[f, c, 4-s, o].  psum1[t_local, (s,f2,o)] then
     holds the contribution of input time t to output time t + s.
   pass 2: shift-matrix matmuls S_s[t, t'] = delta(t = t' + 4 - s)
     contract over the 128 t-partitions to realign the 5 shifted
     contributions onto common output times, accumulating in PSUM, plus a
     rank-1 matmul that adds the bias.  Windows advance by 124 so every
     output column of a window is complete (no cross-window terms).
"""
import os
import sys

for _p in (
    "/root/.axon_site",
    "/root/.axon_site/_ro/trn_rl_repo",
    "/root/.axon_site/_ro/pypackages",
    "/opt/trn_rl_repo",
):
    if os.path.isdir(_p) and _p not in sys.path:
        sys.path.append(_p)

import numpy as np
import ml_dtypes

import concourse.bass as bass  # noqa: F401
import concourse.mybir as mybir
import concourse.tile as tile
from concourse import bacc, bass_utils

TC = 5
B, C, T, F = 4, 64, 1000, 96
O = 2
NCORES = 8
FL = F // NCORES       # 12 freqs per core
NFP = FL // 2          # 6 f-pairs
NTB = 9                # t-windows per b: 8 x stride-124 + tail at 876
WIN = 128
STEP = 124
TPAD = T + 4           # 1004 cols per b in SBUF (4 leading zeros)
NQ = 2 * O             # 4 = (f2, o)
PS2N = 2 * NFP * NTB * NQ   # 432 cols per psum2 bank (2 b's)
F32 = mybir.dt.float32

# pass-1 input dtype: bf16 halves HBM traffic and lets LDWEIGHTS overlap /
# use FWL, at ~2e-3 relative error; fp32 is exact but ~2.5x slower on PE.
DT16 = os.environ.get("KERNEL_DT16", "0") == "1"


def _host_prep(x, weight, b):
    """Full inputs -> per-core input maps (numpy only)."""
    x = np.ascontiguousarray(np.asarray(x, dtype=np.float32))
    weight = np.asarray(weight, dtype=np.float32)
    bias = np.asarray(b, dtype=np.float32)
    x_t = np.ascontiguousarray(x.transpose(3, 1, 0, 2))   # [F,C,B,T]
    w4 = weight.reshape(F, C, TC, O)

    # shift matrices, core-independent: [128, 6*124] (partition-major)
    sm = np.zeros((6, 128, STEP), np.float32)
    for s in range(TC):
        tp = np.arange(STEP)
        sm[s, tp + 4 - s, tp] = 1.0
    sm[5, 0, :] = 1.0       # bias row-broadcast matrix
    sm_h = np.ascontiguousarray(sm.transpose(1, 0, 2).reshape(128, 6 * STEP))

    xdt = ml_dtypes.bfloat16 if DT16 else np.float32
    in_maps = []
    for g in range(NCORES):
        f0 = g * FL
        xg = np.zeros((NFP, 2 * C, B, TPAD), np.float32)
        xg[:, :, :, 4:] = x_t[f0:f0 + FL].reshape(NFP, 2 * C, B, T)
        # -> [3, 128, 2*B*TPAD]: two f-pairs per SBUF tile, partition-major
        xg = np.ascontiguousarray(
            xg.reshape(3, 2, 2 * C, B * TPAD).transpose(0, 2, 1, 3)
        ).reshape(3, 2 * C, 2 * B * TPAD).astype(xdt)

        we = np.zeros((NFP, 128, 20), np.float32)
        w5 = w4[f0:f0 + FL].reshape(NFP, 2, C, TC, O)
        w5s = w5[:, :, :, ::-1, :]                        # tap k -> shift s=4-k
        wev = we.reshape(NFP, 2, C, TC, 2, O)             # fp,f2,c,s,f2',o
        for f2 in range(2):
            wev[:, f2, :, :, f2, :] = w5s[:, f2]
        we_h = np.ascontiguousarray(
            we.transpose(1, 0, 2).reshape(128, NFP * 20)).astype(xdt)

        br = np.zeros((128, PS2N), np.float32)
        v = bias[f0:f0 + FL].reshape(NFP, 2, O)
        arr = np.broadcast_to(v[:, None, :, :], (NFP, NTB, 2, O)).reshape(-1)
        br[0] = np.tile(arr, 2)

        in_maps.append({
            "xin": xg,
            "wexp": we_h,
            "consts": np.ascontiguousarray(np.concatenate([sm_h, br], axis=1)),
        })
    return in_maps


def build_program(nc):
    """Declare DRAM tensors + emit the Tile program."""
    XDT = mybir.dt.bfloat16 if DT16 else F32
    xin = nc.dram_tensor("xin", [3, 128, 2 * B * TPAD], XDT,
                         kind="ExternalInput").ap()
    wexp = nc.dram_tensor("wexp", [128, NFP * 20], XDT,
                          kind="ExternalInput").ap()
    ncst = 6 * STEP + PS2N      # 744 + 432 = 1176
    cst = nc.dram_tensor("consts", [128, ncst], F32, kind="ExternalInput").ap()
    # One DRAM output per b: raw [p, tb, q] window dump, p-major so each
    # partition writes one contiguous 864 B run; the host re-indexes
    # t = tb*124 + p (tb<8) / t = 876 + p (tb=8) and drops overlap rows.
    outs = [nc.dram_tensor(f"out{bb}", [STEP, NTB * FL * O], F32,
                           kind="ExternalOutput").ap() for bb in range(B)]

    with tile.TileContext(nc) as tc:
        from contextlib import ExitStack
        with ExitStack() as ctx:
            const = ctx.enter_context(tc.tile_pool(name="const", bufs=1))
            ps1_pool = ctx.enter_context(
                tc.tile_pool(name="ps1", bufs=6, space="PSUM"))
            ps2_pool = ctx.enter_context(
                tc.tile_pool(name="ps2", bufs=2, space="PSUM"))

            cst_sb = const.tile([128, ncst], F32, name="cst_sb")
            w_all = const.tile([128, NFP * 20], XDT, name="w_all")
            tmp = const.tile([128, B * NFP * NTB * 20], F32, name="tmp")
            stage = const.tile([128, B * NTB * 24], F32, name="stage")
            xt = [const.tile([128, 2 * B * TPAD], XDT, name=f"x_sb{j}")
                  for j in range(3)]
            sm_sb = cst_sb[:, 0:6 * STEP]
            br_sb = cst_sb[:, 6 * STEP:ncst]

            def xs(fp):
                return xt[fp // 2][:, (fp % 2) * B * TPAD:
                                   (fp % 2 + 1) * B * TPAD]

            nc.sync.dma_start(cst_sb[:], cst)
            nc.sync.dma_start(w_all[:], wexp)
            for j in range(3):
                nc.sync.dma_start(xt[j][:], xin[j])

            # pass 1
            for fp in range(NFP):
                for bb in range(B):
                    ps1 = ps1_pool.tile([128, NTB * 20], F32, tag="ps1")
                    for tb in range(NTB):
                        c0 = tb * STEP if tb < 8 else TPAD - WIN
                        nc.tensor.matmul(
                            ps1[:, tb * 20:(tb + 1) * 20],
                            lhsT=xs(fp)[:, bb * TPAD + c0: bb * TPAD + c0 + WIN],
                            rhs=w_all[:, fp * 20:(fp + 1) * 20],
                            start=True, stop=True)
                    # single engine (DVE) keeps `tmp` consumers at one wait sem
                    nc.vector.tensor_copy(
                        tmp[:, bb * 1080 + fp * 180: bb * 1080 + (fp + 1) * 180],
                        ps1[:])

            # pass 2: tmp col = b*1080 + (fp*9+tb)*20 + s*4 + (f2*2+o)
            tmp4 = tmp[:].rearrange("p (b m u) -> p b m u", b=B, m=NFP * NTB)
            for bank in range(2):
                ps2 = ps2_pool.tile([STEP, PS2N], F32, tag="ps2")
                nc.tensor.matmul(ps2[:], lhsT=sm_sb[:, 5 * STEP:6 * STEP],
                                 rhs=br_sb, start=True, stop=False)
                for s in range(TC):
                    rhs = tmp4[:, 2 * bank:2 * bank + 2, :, s * NQ:(s + 1) * NQ]
                    nc.tensor.matmul(ps2[:], lhsT=sm_sb[:, s * STEP:(s + 1) * STEP],
                                     rhs=rhs, start=False, stop=(s == TC - 1))
                # psum2 col = bl*216 + fp*36 + tb*4 + q -> stage col tb*24+fp*4+q
                for bl in range(2):
                    bb = 2 * bank + bl
                    src = ps2[:, bl * 216:(bl + 1) * 216].rearrange(
                        "p (f m q) -> p m f q", f=NFP, m=NTB)
                    nc.vector.tensor_copy(
                        stage[:STEP, bb * 216:(bb + 1) * 216].rearrange(
                            "p (m f q) -> p m f q", m=NTB, f=NFP), src)

            for bb in range(B):
                nc.sync.dma_start(outs[bb],
                                  stage[:STEP, bb * 216:(bb + 1) * 216])
    return "out"


_CACHED = {}


def _get_nc():
    if "nc" not in _CACHED:
        # bacc.Bacc (not bass.Bass): its compile() pass legalizes multi-wait
        # instructions onto InstEventSemaphore (1 wait/inst ISA limit)
        nc = bacc.Bacc("TRN2", target_bir_lowering=False, debug=False,
                       num_devices=NCORES)
        build_program(nc)
        nc.compile()
        _CACHED["nc"] = nc
    return _CACHED["nc"]


def _gather(results):
    full = np.empty((B, O, T, F), np.float32)
    co = np.empty((B, T, FL * O), np.float32)
    for g in range(NCORES):
        for bb in range(B):
            arr = results[g][f"out{bb}"].reshape(STEP, NTB, FL * O)
            co[bb, :8 * STEP] = arr[:, :8].transpose(1, 0, 2).reshape(
                8 * STEP, FL * O)
            co[bb, 8 * STEP:] = arr[STEP - (T - 8 * STEP):, 8]
        full[:, :, :, g * FL:(g + 1) * FL] = \
            co.reshape(B, T, FL, O).transpose(0, 3, 1, 2)
    return full


def kernel(x, weight, b, **run_kwargs):
    in_maps = _host_prep(x, weight, b)
    nc = _get_nc()
    res = bass_utils.run_bass_kernel_spmd(
        nc, in_maps, core_ids=list(range(NCORES)), **run_kwargs)
    out = _gather(res.results)
    if run_kwargs:
        return out, res
    return out
